# revision 11
# baseline (speedup 1.0000x reference)
"""Trainium2 Bass kernel for nn_ConstraintsModule.

Reference math:
    m = preds[:, atoms]                                   # [B, N]
    body_rev[b,c,j] = pos_body[c,j] + m[b,j]*(neg_body-pos_body)[c,j]
    body_min[b,c]   = 1 - max_j body_rev[b,c,j]
    lb[b,n] = max_c body_min[b,c]*pos_head[c,n]
    ub[b,n] = 1 - max_c body_min[b,c]*neg_head[c,n]
    updated = clamp(m, min(lb,ub), max(lb,ub))
    out = preds with columns `atoms` replaced by updated

Device computes, per (batch row, head-atom, sign) "bin":
    bound[bin] = max over the bin's constraints c of
                 body_min[c] = min(m_{j in pos(c)}, (1-m)_{j in neg(c)}, 1)
via host-packed rows [m_pos..., (1-m)_neg..., 1.0 pads] (min-space form)
in bf16 (min/max never create values, so only the initial bf16 rounding
matters; measured rel err 3.9e-3 vs the 2e-2 gate).  The tiny elementwise
clamp (lb/ub vs m, O(B*N)) plus gather/scatter runs on the host, which
also owns the column -> (atom, sign) mapping, so device column order is
free to follow slot order.

Device structure (all compute on DVE; only DVE can reduce on this target):
  * G is split into uniform-width tier runs.  Each run is first narrowed
    by tensor_tensor min "fold" levels (bf16 2x mode, halving the width),
    then finished by tensor_reduce tails:
      - size-1 bins (the majority), packed first in the run: one reduce
        straight into output columns (body+head fused);
      - size>=2 bins, grouped by bucketed size class: one reduce per
        (run, class) into a bmin scratch arranged class-major.
  * One segment max-reduce per size class (over ALL runs/sides at once)
    produces the remaining output columns.

Sharding: bins follow their head atom; atom-sides are dealt round-robin
to the 8 cores so all cores share one SPMD program (counts padded to
ceil(n/8), dummy slots = all-zero rows -> bound 0, ignored by host).
"""

import sys
from collections import defaultdict
from contextlib import ExitStack

import numpy as np

if "/opt/trn_rl_repo" not in sys.path:
    sys.path.insert(0, "/opt/trn_rl_repo")

import ml_dtypes

import concourse.bacc as bacc
import concourse.tile as tile
from concourse import mybir
from concourse.bass_utils import run_bass_kernel_spmd

B = 128
C = 1024
N = 512
NCORES = 8

TIERS = (24, 32, 38)      # slot width tiers (per atom-side max constraint width)
SBUCK = (1, 2, 4, 8)      # bin-size buckets (1 kept exact -> fused)
CHUNK_ELEMS = 1500        # target per-partition elems per DMA piece
FIRST_PIECE_ELEMS = 450   # small first piece per run for early vector start
FOLD_MIN_ELEMS = 800      # keep folding while slots*width exceeds this

_TRACE = False
_LAST_RESULTS = None
_PROGRAM_CACHE: dict = {}


def _bucket(x):
    for v in SBUCK:
        if x <= v:
            return v
    raise ValueError(f"bin size {x} > {SBUCK[-1]}")


def _build_structure(bins):
    """bins: list of (side, atom, [cids], tier_idx, size).

    Layout:
      slot space: [run per tier] each = [fused bins][class-2 bins][class-4]..
      bmin space: class-major: [class2: run0, run1..][class4: ..]
      col space:  [run0 fused][run1 fused]..[class2 cols][class4 cols]..
    """
    gat = defaultdict(list)
    for side, atom, cids, t, s in bins:
        gat[(t, 1 if s == 1 else _bucket(s), side)].append((atom, cids))

    groups = []
    for (t, sb, side), members in sorted(gat.items()):
        cnt = -(-len(members) // NCORES)
        groups.append(dict(
            tier=t, sb=sb, side=side, cnt=cnt, members=members,
            w=TIERS[t], nslots=cnt * sb,
        ))

    soff = 0
    runs = []
    for t in range(len(TIERS)):
        tg = [g for g in groups if g["tier"] == t]
        if not tg:
            continue
        r = dict(t=t, w=TIERS[t], lo=soff, nfused=0, classes={})
        for g in tg:          # sorted: sb=1 first, then sb ascending
            g["soff"] = soff
            soff += g["nslots"]
            if g["sb"] == 1:
                r["nfused"] += g["cnt"]
            else:
                lo, n = r["classes"].get(g["sb"], (None, 0))
                if lo is None:
                    lo = g["soff"]
                r["classes"][g["sb"]] = (lo, n + g["nslots"])
        r["hi"] = soff
        runs.append(r)
    nslots = soff

    bptr = 0
    classes = {}
    for q in sorted({g["sb"] for g in groups if g["sb"] > 1}):
        chunks = []
        for ri, r in enumerate(runs):
            if q in r["classes"]:
                lo, n = r["classes"][q]
                chunks.append((ri, lo, n, bptr))
                bptr += n
        classes[q] = dict(q=q, chunks=chunks, boff0=chunks[0][3],
                          total=sum(c[2] for c in chunks))
    nbmin = bptr

    col = 0
    for r in runs:
        r["fcol"] = col
        col += r["nfused"]
    for q in sorted(classes):
        classes[q]["col"] = col
        col += classes[q]["total"] // q
    ncols = col

    for g in groups:
        ri = next(i for i, r in enumerate(runs) if r["t"] == g["tier"])
        r = runs[ri]
        if g["sb"] == 1:
            g["col"] = r["fcol"] + (g["soff"] - r["lo"])
        else:
            cl = classes[g["sb"]]
            ch = next(c for c in cl["chunks"] if c[0] == ri)
            boff = ch[3] + (g["soff"] - ch[1])
            g["boff"] = boff
            g["col"] = cl["col"] + (boff - cl["boff0"]) // g["sb"]

    core_members = [[] for _ in range(NCORES)]
    for gi, g in enumerate(groups):
        for j, (atom, cids) in enumerate(g["members"]):
            core_members[j % NCORES].append((gi, j // NCORES, atom, cids))

    return dict(groups=groups, runs=runs, classes=classes, nslots=nslots,
                nbmin=nbmin, ncols=ncols, core_members=core_members)


def _plan_dma(runs):
    """Per run: one small first piece (early vector start), then ~CHUNK_ELEMS
    pieces.  Returned in waves: every run's first piece, then the rest, so
    each run's data starts landing early on the two queues."""
    per_run = []
    for r in runs:
        n = r["hi"] - r["lo"]
        w = r["w"]
        lst = []
        first = min(max(FIRST_PIECE_ELEMS // w, 4), n)
        lst.append((r["lo"], r["lo"] + first, w))
        s = r["lo"] + first
        rem = r["hi"] - s
        if rem > 0:
            k = max(1, round(rem * w / CHUNK_ELEMS))
            step = -(-rem // k)
            while s < r["hi"]:
                lst.append((s, min(s + step, r["hi"]), w))
                s += step
        per_run.append(lst)
    pieces = []
    wave = 0
    while any(wave < len(lst) for lst in per_run):
        for lst in per_run:
            if wave < len(lst):
                pieces.append(lst[wave])
        wave += 1
    return pieces


def _build_program(skey, st, pieces):
    if skey in _PROGRAM_CACHE:
        return _PROGRAM_CACHE[skey]
    dt = mybir.dt
    bf = dt.bfloat16
    ncols = st["ncols"]
    nbmin = max(st["nbmin"], 1)

    nc = bacc.Bacc(
        "TRN2", target_bir_lowering=False, debug=False, enable_partition_id=False
    )
    c_ds = [
        nc.dram_tensor(f"c{i}", [B, (s1 - s0) * w], bf, kind="ExternalInput")
        for i, (s0, s1, w) in enumerate(pieces)
    ]
    out_d = nc.dram_tensor("bounds", [B, ncols], bf, kind="ExternalOutput")

    with ExitStack() as ctx:
        tc = ctx.enter_context(tile.TileContext(nc))
        pool = ctx.enter_context(tc.tile_pool(name="main", bufs=1))

        bounds = pool.tile([B, ncols], bf, tag="bounds")
        bmin = pool.tile([B, nbmin], bf, tag="bmin")

        run_tiles = []
        for r in st["runs"]:
            rt = pool.tile([B, (r["hi"] - r["lo"]) * r["w"]], bf,
                           name=f"run{r['lo']}", tag=f"run{r['lo']}")
            run_tiles.append(rt)

        dmaq = [nc.sync, nc.scalar]
        for i, (s0, s1, w) in enumerate(pieces):
            for r, rt in zip(st["runs"], run_tiles):
                if r["lo"] <= s0 and s1 <= r["hi"]:
                    dmaq[i % 2].dma_start(
                        rt[:, (s0 - r["lo"]) * w : (s1 - r["lo"]) * w],
                        c_ds[i].ap(),
                    )
                    break

        for ri, (r, rt) in enumerate(zip(st["runs"], run_tiles)):
            nrs = r["hi"] - r["lo"]
            w = r["w"]
            cur = rt[:].rearrange("p (s w) -> p s w", w=w)
            curw = w
            scratch = [None, None]
            pp = 0
            first_level = True
            while curw > 2 and nrs * curw > FOLD_MIN_ELEMS:
                nh = (curw + 1) // 2
                if scratch[pp] is None:
                    scratch[pp] = pool.tile(
                        [B, nrs * nh], bf,
                        name=f"fs{r['lo']}_{pp}", tag=f"fs{r['lo']}_{pp}",
                    )
                nxt = scratch[pp][:, 0 : nrs * nh].rearrange(
                    "p (s w) -> p s w", w=nh
                )
                if first_level:
                    # split by DMA piece so folding starts on the first piece
                    for (s0, s1, pw) in pieces:
                        if s0 < r["lo"] or s1 > r["hi"]:
                            continue
                        a, b = s0 - r["lo"], s1 - r["lo"]
                        nc.vector.tensor_tensor(
                            nxt[:, a:b, :], cur[:, a:b, 0:nh],
                            cur[:, a:b, curw - nh : curw],
                            op=mybir.AluOpType.min,
                        )
                    first_level = False
                else:
                    nc.vector.tensor_tensor(
                        nxt, cur[:, :, 0:nh], cur[:, :, curw - nh : curw],
                        op=mybir.AluOpType.min,
                    )
                cur, curw = nxt, nh
                pp ^= 1
            if r["nfused"]:
                nc.vector.tensor_reduce(
                    bounds[:, r["fcol"] : r["fcol"] + r["nfused"]],
                    cur[:, 0 : r["nfused"], :],
                    axis=mybir.AxisListType.X, op=mybir.AluOpType.min,
                )
            for q in sorted(st["classes"]):
                for (cri, slot_lo, n, boff) in st["classes"][q]["chunks"]:
                    if cri != ri:
                        continue
                    rel = slot_lo - r["lo"]
                    nc.vector.tensor_reduce(
                        bmin[:, boff : boff + n],
                        cur[:, rel : rel + n, :],
                        axis=mybir.AxisListType.X, op=mybir.AluOpType.min,
                    )

        fused_tot = sum(r["nfused"] for r in st["runs"])
        if fused_tot:
            # fused columns are [0, fused_tot); ship them while heads run
            nc.scalar.dma_start(
                out_d.ap()[:, 0:fused_tot], bounds[:, 0:fused_tot]
            )

        for q in sorted(st["classes"]):
            cl = st["classes"][q]
            ncl = cl["total"] // q
            seg = bmin[:, cl["boff0"] : cl["boff0"] + cl["total"]].rearrange(
                "p (a q) -> p a q", q=q
            )
            nc.vector.tensor_reduce(
                bounds[:, cl["col"] : cl["col"] + ncl], seg,
                axis=mybir.AxisListType.X, op=mybir.AluOpType.max,
            )

        if fused_tot < ncols:
            nc.sync.dma_start(
                out_d.ap()[:, fused_tot:ncols], bounds[:, fused_tot:ncols]
            )

    nc.compile()
    _PROGRAM_CACHE[skey] = nc
    return nc


def kernel(preds, pos_head, neg_head, pos_body, neg_body, atoms):
    global _LAST_RESULTS
    preds = np.ascontiguousarray(np.asarray(preds, dtype=np.float32))
    pos_head = np.asarray(pos_head)
    neg_head = np.asarray(neg_head)
    pos_body = np.asarray(pos_body)
    neg_body = np.asarray(neg_body)
    atoms_np = np.asarray(atoms).astype(np.int64)

    m = np.ascontiguousarray(preds[:, atoms_np].astype(np.float32))  # [B, N]
    one_m = np.float32(1.0) - m
    # m_ext columns: [0..N) m, [N..2N) 1-m, 2N: 1.0 (pad), 2N+1: 0.0 (dummy)
    m_ext = np.concatenate(
        [m, one_m, np.ones((B, 1), np.float32), np.zeros((B, 1), np.float32)],
        axis=1,
    )
    m_ext_bf = m_ext.astype(ml_dtypes.bfloat16)
    PAD1, PAD0 = 2 * N, 2 * N + 1

    pb = pos_body != 0
    nb_ = neg_body != 0
    k_c = (pb.sum(1) + nb_.sum(1)).astype(np.int64)
    body_js = [
        (np.nonzero(pb[c])[0], np.nonzero(nb_[c])[0]) for c in range(C)
    ]

    ph_atom = pos_head.argmax(1)
    ph_has = pos_head.max(1) > 0
    nh_atom = neg_head.argmax(1)
    nh_has = neg_head.max(1) > 0
    pos_bins = [[] for _ in range(N)]
    neg_bins = [[] for _ in range(N)]
    for c in np.nonzero(ph_has)[0]:
        pos_bins[ph_atom[c]].append(int(c))
    for c in np.nonzero(nh_has)[0]:
        neg_bins[nh_atom[c]].append(int(c))

    bins = []
    for n in range(N):
        for side, lst in (("P", pos_bins[n]), ("N", neg_bins[n])):
            if lst:
                kmax = max(k_c[c] for c in lst)
                t = next(i for i, w in enumerate(TIERS) if kmax <= w)
                bins.append((side, n, lst, t, len(lst)))

    st = _build_structure(bins)
    pieces = _plan_dma(st["runs"])
    skey = (
        tuple((g["tier"], g["side"], g["sb"], g["cnt"], g["col"], g["soff"],
               g.get("boff", -1)) for g in st["groups"]),
        tuple(pieces), st["ncols"],
    )
    nc = _build_program(skey, st, pieces)

    groups = st["groups"]
    in_maps = []
    percore_maps = []   # (cols, atoms, is_pos)
    for core in range(NCORES):
        idx = np.full((max(st["nslots"], 1), max(TIERS)), PAD0, np.int32)
        cl_, at_, sd_ = [], [], []
        for gi, pos_in_g, atom, cids in st["core_members"][core]:
            g = groups[gi]
            w = g["w"]
            base = g["soff"] + pos_in_g * g["sb"]
            for l, cid in enumerate(cids):
                jp, jn = body_js[cid]
                row = idx[base + l]
                row[: jp.size] = jp
                row[jp.size : jp.size + jn.size] = N + jn
                row[jp.size + jn.size : w] = PAD1
            cl_.append(g["col"] + pos_in_g)
            at_.append(atom)
            sd_.append(g["side"] == "P")
        im = {}
        for i, (s0, s1, w) in enumerate(pieces):
            im[f"c{i}"] = np.ascontiguousarray(m_ext_bf[:, idx[s0:s1, :w].ravel()])
        in_maps.append(im)
        percore_maps.append((np.array(cl_, np.int64), np.array(at_, np.int64),
                             np.array(sd_, bool)))

    res = run_bass_kernel_spmd(
        nc, in_maps, core_ids=list(range(NCORES)), trace=_TRACE
    )
    _LAST_RESULTS = res

    lb = np.zeros((B, N), np.float32)
    ubm = np.zeros((B, N), np.float32)
    for core in range(NCORES):
        bounds = np.asarray(res.results[core]["bounds"]).astype(np.float32)
        cols, ats, isp = percore_maps[core]
        if len(cols):
            if isp.any():
                lb[:, ats[isp]] = bounds[:, cols[isp]]
            if (~isp).any():
                ubm[:, ats[~isp]] = bounds[:, cols[~isp]]
    ub = np.float32(1.0) - ubm
    lo = np.minimum(lb, ub)
    hi = np.maximum(lb, ub)
    upd = np.maximum(lo, np.minimum(hi, m))
    out = preds.copy()
    out[:, atoms_np] = upd
    return out


# revision 13
# speedup vs baseline: 1.0897x; 1.0897x over previous
"""Trainium2 Bass kernel for nn_ConstraintsModule.

Reference math:
    m = preds[:, atoms]                                   # [B, N]
    body_rev[b,c,j] = pos_body[c,j] + m[b,j]*(neg_body-pos_body)[c,j]
    body_min[b,c]   = 1 - max_j body_rev[b,c,j]
    lb[b,n] = max_c body_min[b,c]*pos_head[c,n]
    ub[b,n] = 1 - max_c body_min[b,c]*neg_head[c,n]
    updated = clamp(m, min(lb,ub), max(lb,ub))
    out = preds with columns `atoms` replaced by updated

The dominant compute is the per-constraint reduction (the sharding hint's
"per-constraint max-reduction"), rewritten in min-space:
    body_min[b,c] = min(m_{j in pos(c)}, (1-m)_{j in neg(c)}, 1)
The device computes exactly this: the host packs, per constraint, a row
[m_pos..., (1-m)_neg..., 1.0 pads] in bf16 (min/max never create values,
so only the initial bf16 rounding matters; measured rel err 3.9e-3 vs
the 2e-2 gate) and the device min-reduces every row.  Constraints are
sorted into uniform-width tier runs; each run is narrowed with
tensor_tensor min fold levels (bf16 2x DVE mode) and finished with one
tensor_reduce - the fold-vs-reduce depth per run comes from a small DP
over the measured DVE rates.  The first fold level is split per DMA
piece so compute starts as soon as the first piece lands.

Per the hint ("no communication needed until the final gather/scatter
back into preds"), the remaining O(B*(C+N)) elementwise epilogue - the
per-head-atom max over a handful of body_min values, the 1-x flips, the
clamp, and the scatter - runs on the host.

Sharding: each width class is dealt round-robin across the 8 cores
(constraint j -> core j%8), so all cores run one SPMD program with
identical shapes; per-core trailing dummy slots are ignored by the host.
"""

import sys
from functools import lru_cache
from contextlib import ExitStack

import numpy as np

if "/opt/trn_rl_repo" not in sys.path:
    sys.path.insert(0, "/opt/trn_rl_repo")

import ml_dtypes

import concourse.bacc as bacc
import concourse.tile as tile
from concourse import mybir
from concourse.bass_utils import run_bass_kernel_spmd

B = 128
C = 1024
N = 512
NCORES = 8

TIER_BOUNDS = (28,)       # interior tier boundaries; last tier = max k
CHUNK_ELEMS = 1600        # target per-partition elems per DMA piece
FIRST_PIECE_ELEMS = 450   # small first piece per run for early vector start

OP_FIX, R_RED, R_TT = 120.0, 1.042, 0.52   # DVE cost model (ns, ns/elem)

_TRACE = False
_LAST_RESULTS = None
_PROGRAM_CACHE: dict = {}


def _fold_plan(s, w):
    """Widths sequence for min-reducing [s slots, w] on DVE.
    Returns (levels, final): fold through `levels` widths, then one
    final op at the remaining width ('tt2' pairwise if width 2)."""

    @lru_cache(None)
    def dp(cw):
        stop_cost = OP_FIX + R_RED * cw * s
        if cw <= 2:
            return (OP_FIX + R_RED * s, ())
        nh = (cw + 1) // 2
        sub_cost, sub = dp(nh)
        fold_cost = OP_FIX + R_TT * nh * s + sub_cost
        if stop_cost <= fold_cost:
            return (stop_cost, None)        # None -> reduce at cw
        return (fold_cost, (nh,) + (sub if sub is not None else ()))

    cost, seq = dp(w)
    if seq is None:
        return (), w
    # seq is the chain of widths; find where it stops
    levels = []
    cw = w
    while True:
        if cw <= 2:
            break
        nh = (cw + 1) // 2
        c_stop = OP_FIX + R_RED * cw * s
        c_sub, _ = dp(nh)
        if c_stop <= OP_FIX + R_TT * nh * s + c_sub:
            break
        levels.append(nh)
        cw = nh
    return tuple(levels), cw


def _build_structure(k_c):
    tiers = tuple(sorted(TIER_BOUNDS)) + (int(max(k_c)),)
    runs = []
    soff = 0
    lo = 0
    for wt in tiers:
        cids = np.where((k_c > lo) & (k_c <= wt))[0]
        lo = wt
        if len(cids) == 0:
            continue
        cnt = -(-len(cids) // NCORES)
        runs.append(dict(w=int(wt), cids=cids, cnt=cnt, lo=soff,
                         hi=soff + cnt, elems=cnt * int(wt)))
        soff += cnt
    runs.sort(key=lambda r: -r["elems"])   # big runs first (stream order)
    off = 0
    for r in runs:
        r["lo"], r["hi"] = off, off + r["cnt"]
        off += r["cnt"]
        r["plan"] = _fold_plan(r["cnt"], r["w"])
    return dict(runs=runs, nslots=off)


def _plan_dma(runs):
    """Waves of pieces: every run's small first piece, then the rest."""
    per_run = []
    for r in runs:
        n = r["cnt"]
        w = r["w"]
        lst = []
        first = min(max(FIRST_PIECE_ELEMS // w, 4), n)
        lst.append((r["lo"], r["lo"] + first, w))
        s = r["lo"] + first
        rem = r["hi"] - s
        if rem > 0:
            kk = max(1, round(rem * w / CHUNK_ELEMS))
            step = -(-rem // kk)
            while s < r["hi"]:
                lst.append((s, min(s + step, r["hi"]), w))
                s += step
        per_run.append(lst)
    pieces = []
    wave = 0
    while any(wave < len(lst) for lst in per_run):
        for lst in per_run:
            if wave < len(lst):
                pieces.append(lst[wave])
        wave += 1
    return pieces


def _build_program(skey, st, pieces):
    if skey in _PROGRAM_CACHE:
        return _PROGRAM_CACHE[skey]
    dt = mybir.dt
    bf = dt.bfloat16
    nslots = st["nslots"]

    nc = bacc.Bacc(
        "TRN2", target_bir_lowering=False, debug=False, enable_partition_id=False
    )
    c_ds = [
        nc.dram_tensor(f"c{i}", [B, (s1 - s0) * w], bf, kind="ExternalInput")
        for i, (s0, s1, w) in enumerate(pieces)
    ]
    out_d = nc.dram_tensor("bmin", [B, nslots], bf, kind="ExternalOutput")

    with ExitStack() as ctx:
        tc = ctx.enter_context(tile.TileContext(nc))
        pool = ctx.enter_context(tc.tile_pool(name="main", bufs=1))

        bmin = pool.tile([B, nslots], bf, tag="bmin")

        run_tiles = []
        for r in st["runs"]:
            rt = pool.tile([B, r["cnt"] * r["w"]], bf,
                           name=f"run{r['lo']}", tag=f"run{r['lo']}")
            run_tiles.append(rt)

        dmaq = [nc.sync, nc.scalar]
        for i, (s0, s1, w) in enumerate(pieces):
            for r, rt in zip(st["runs"], run_tiles):
                if r["lo"] <= s0 and s1 <= r["hi"]:
                    dmaq[i % 2].dma_start(
                        rt[:, (s0 - r["lo"]) * w : (s1 - r["lo"]) * w],
                        c_ds[i].ap(),
                    )
                    break

        for r, rt in zip(st["runs"], run_tiles):
            s = r["cnt"]
            w = r["w"]
            levels, final_w = r["plan"]
            cur = rt[:].rearrange("p (s w) -> p s w", w=w)
            curw = w
            scratch = [None, None]
            pp = 0
            first_level = True
            for nh in levels:
                if scratch[pp] is None:
                    scratch[pp] = pool.tile(
                        [B, s * nh], bf,
                        name=f"fs{r['lo']}_{pp}", tag=f"fs{r['lo']}_{pp}",
                    )
                nxt = scratch[pp][:, 0 : s * nh].rearrange(
                    "p (s w) -> p s w", w=nh
                )
                if first_level:
                    for (p0, p1, pw) in pieces:
                        if p0 < r["lo"] or p1 > r["hi"]:
                            continue
                        a, b = p0 - r["lo"], p1 - r["lo"]
                        nc.vector.tensor_tensor(
                            nxt[:, a:b, :], cur[:, a:b, 0:nh],
                            cur[:, a:b, curw - nh : curw],
                            op=mybir.AluOpType.min,
                        )
                    first_level = False
                else:
                    nc.vector.tensor_tensor(
                        nxt, cur[:, :, 0:nh], cur[:, :, curw - nh : curw],
                        op=mybir.AluOpType.min,
                    )
                cur, curw = nxt, nh
                pp ^= 1
            dst = bmin[:, r["lo"] : r["hi"]]
            if curw == 2:
                nc.vector.tensor_tensor(
                    dst, cur[:, :, 0:1], cur[:, :, 1:2], op=mybir.AluOpType.min
                )
            elif first_level:
                # no folds at all: reduce, split per piece for DMA overlap
                for (p0, p1, pw) in pieces:
                    if p0 < r["lo"] or p1 > r["hi"]:
                        continue
                    a, b = p0 - r["lo"], p1 - r["lo"]
                    nc.vector.tensor_reduce(
                        bmin[:, p0:p1], cur[:, a:b, :],
                        axis=mybir.AxisListType.X, op=mybir.AluOpType.min,
                    )
            else:
                nc.vector.tensor_reduce(
                    dst, cur, axis=mybir.AxisListType.X, op=mybir.AluOpType.min
                )
            # ship this run's bmin as soon as it is done
            nc.sync.dma_start(out_d.ap()[:, r["lo"] : r["hi"]], dst)

    nc.compile()
    _PROGRAM_CACHE[skey] = nc
    return nc


def kernel(preds, pos_head, neg_head, pos_body, neg_body, atoms):
    global _LAST_RESULTS
    preds = np.ascontiguousarray(np.asarray(preds, dtype=np.float32))
    pos_head = np.asarray(pos_head)
    neg_head = np.asarray(neg_head)
    pos_body = np.asarray(pos_body)
    neg_body = np.asarray(neg_body)
    atoms_np = np.asarray(atoms).astype(np.int64)

    m = np.ascontiguousarray(preds[:, atoms_np].astype(np.float32))  # [B, N]
    one_m = np.float32(1.0) - m
    # m_ext columns: [0..N) m, [N..2N) 1-m, 2N: 1.0 (pad), 2N+1: 0.0 (dummy)
    m_ext = np.concatenate(
        [m, one_m, np.ones((B, 1), np.float32), np.zeros((B, 1), np.float32)],
        axis=1,
    )
    m_ext_bf = m_ext.astype(ml_dtypes.bfloat16)
    PAD1, PAD0 = 2 * N, 2 * N + 1

    pb = pos_body != 0
    nb_ = neg_body != 0
    k_c = (pb.sum(1) + nb_.sum(1)).astype(np.int64)

    st = _build_structure(k_c)
    pieces = _plan_dma(st["runs"])
    skey = (
        tuple((r["w"], r["cnt"], r["lo"], r["plan"]) for r in st["runs"]),
        tuple(pieces), st["nslots"],
    )
    nc = _build_program(skey, st, pieces)

    # pack per-core index maps (slot row -> m_ext columns)
    idx = np.full((NCORES, max(st["nslots"], 1), max(r["w"] for r in st["runs"])),
                  PAD0, np.int32)
    for r in st["runs"]:
        w = r["w"]
        for j, cid in enumerate(r["cids"]):
            core = j % NCORES
            slot = r["lo"] + j // NCORES
            jp = np.nonzero(pb[cid])[0]
            jn = np.nonzero(nb_[cid])[0]
            row = idx[core, slot]
            row[: jp.size] = jp
            row[jp.size : jp.size + jn.size] = N + jn
            row[jp.size + jn.size : w] = PAD1
    in_maps = []
    for core in range(NCORES):
        im = {}
        for i, (s0, s1, w) in enumerate(pieces):
            im[f"c{i}"] = np.ascontiguousarray(
                m_ext_bf[:, idx[core, s0:s1, :w].ravel()]
            )
        in_maps.append(im)

    res = run_bass_kernel_spmd(
        nc, in_maps, core_ids=list(range(NCORES)), trace=_TRACE
    )
    _LAST_RESULTS = res

    # reassemble per-constraint body_min (empty-body constraints -> 1.0)
    bm = np.ones((B, C), np.float32)
    outs = [np.asarray(res.results[core]["bmin"]).astype(np.float32)
            for core in range(NCORES)]
    for r in st["runs"]:
        for core in range(NCORES):
            mine = r["cids"][core::NCORES]
            if len(mine):
                bm[:, mine] = outs[core][:, r["lo"] : r["lo"] + len(mine)]

    # host epilogue: per-head-atom max, 1-x, clamp, scatter
    ph_atom = pos_head.argmax(1)
    ph_has = pos_head.max(1) > 0
    nh_atom = neg_head.argmax(1)
    nh_has = neg_head.max(1) > 0
    lb = np.zeros((B, N), np.float32)
    ubm = np.zeros((B, N), np.float32)
    for has, hatom, dst in ((ph_has, ph_atom, lb), (nh_has, nh_atom, ubm)):
        cs = np.nonzero(has)[0]
        if len(cs) == 0:
            continue
        order = np.argsort(hatom[cs], kind="stable")
        cs = cs[order]
        a_sorted = hatom[cs]
        starts = np.nonzero(np.r_[True, a_sorted[1:] != a_sorted[:-1]])[0]
        vals = np.maximum.reduceat(bm[:, cs], starts, axis=1)
        dst[:, a_sorted[starts]] = vals
    ub = np.float32(1.0) - ubm
    lo = np.minimum(lb, ub)
    hi = np.maximum(lb, ub)
    upd = np.maximum(lo, np.minimum(hi, m))
    out = preds.copy()
    out[:, atoms_np] = upd
    return out
